# revision 13
# baseline (speedup 1.0000x reference)
"""Trainium2 Bass kernel for MatrixOdeGradientDescentModel.

Reference computation (B=4096, DZ=512, H=2048, DY=10, n_steps=64):
    z = x; repeat n_steps: z += dt * z @ A.T          (dt = 1/n_steps)
    y = relu(z @ W1.T + b1) @ W2.T + b2

Algebraic rewrite: the Euler loop is linear, so
    z_final = x @ (M^T)^n  with  M = I + dt*A.
We compute W := M^T = I + dt*A^T by repeated squaring on the *deviation*
D_k := W^(2^k) - I (avoids precision loss from the identity's magnitude):
    D_{k+1} = 2*D_k + D_k @ D_k
maintaining the pair (D_k, T_k=D_k^T) so no on-device transposes are needed:
    D@D = matmul(lhsT=T, rhs=D),   (D@D)^T = matmul(lhsT=D, rhs=T)
then zT = xT + D_chain applied to xT per set bit of n (binary exponentiation).

Sharding: data-parallel over batch. Each of the 8 cores gets 512 rows of x;
A/W1/W2 replicated; no cross-core communication.

Matmuls run in float32r (TF32-like, 4x faster than fp32 on the PE) with fp32
PSUM accumulation; the error-compensated deviation chain keeps the end-to-end
relative error at the ~1e-4 level.
"""

import os

import numpy as np

import concourse.bacc as bacc
import concourse.mybir as mybir
import concourse.tile as tile
from concourse.bass_utils import run_bass_kernel_spmd

P = 128
B, DZ, H, DY = 4096, 512, 2048, 10
NCORES = 8
BC = B // NCORES          # 512 rows per core
DT = DZ // P              # 4 k-tiles over DZ
HT = H // P               # 16 m-tiles over H

f32 = mybir.dt.float32
f32r = mybir.dt.float32r

_BUILD_CACHE = {}


def _emit_mm_set(nc, psum_pool, lhsT_tile, rhs_tile, evict, n_mt=DT):
    """One [512,512]-ish matmul set: for each output row-block mt, accumulate
    over DT k-tiles into PSUM and call evict(mt, psum_ap)."""
    for mt in range(n_mt):
        ps = psum_pool.tile([P, BC], f32, tag="ps")
        for kt in range(DT):
            nc.tensor.matmul(
                ps[:],
                lhsT_tile[:, kt, mt * P:(mt + 1) * P],
                rhs_tile[:, kt, :],
                start=(kt == 0),
                stop=(kt == DT - 1),
            )
        evict(mt, ps)


def _build(n_steps: int):
    """Build + compile the Bass module for a given n_steps."""
    n = int(n_steps)
    assert n >= 0
    nc = bacc.Bacc("TRN2", target_bir_lowering=False, debug=False,
                   enable_asserts=False, num_devices=NCORES)

    # f32r-declared DRAM inputs carry raw fp32 bytes; the PE rounds internally
    # (verified bit-identical to an explicit cast) so plain HWDGE DMA works.
    xt_d = nc.dram_tensor("xt", [P, DT * BC], f32, kind="ExternalInput")
    xtr_d = nc.dram_tensor("xtr", [P, DT * BC], f32r, kind="ExternalInput")
    d0_d = nc.dram_tensor("d0", [P, DT * DZ], f32r, kind="ExternalInput")
    t0_d = nc.dram_tensor("t0", [P, DT * DZ], f32r, kind="ExternalInput")
    w1t_d = nc.dram_tensor("w1t", [P, DT * H], f32r, kind="ExternalInput")
    b1t_d = nc.dram_tensor("b1t", [P, HT], f32, kind="ExternalInput")
    w2t_d = nc.dram_tensor("w2t", [P, HT * DY], f32r, kind="ExternalInput")
    b2t_d = nc.dram_tensor("b2t", [DY, 1], f32, kind="ExternalInput")
    ident_d = nc.dram_tensor("ident", [P, P], f32, kind="ExternalInput")
    y_d = nc.dram_tensor("y", [BC, DY], f32, kind="ExternalOutput")

    mult = mybir.AluOpType.mult
    add = mybir.AluOpType.add

    with tile.TileContext(nc) as tc:
        napply = bin(n).count("1") if n > 0 else 0
        with (
            tc.tile_pool(name="const", bufs=1) as const_pool,
            tc.tile_pool(name="weights", bufs=1) as w_pool,
            tc.tile_pool(name="chain", bufs=2) as chain_pool,
            tc.tile_pool(name="accp", bufs=min(napply, 2) or 1) as acc_pool,
            tc.tile_pool(name="acts", bufs=1) as act_pool,
            tc.tile_pool(name="out", bufs=2) as out_pool,
            tc.tile_pool(name="psum", bufs=7, space="PSUM") as psum_pool,
            tc.tile_pool(name="psum_y", bufs=1, space="PSUM") as psum_y_pool,
        ):
            # ---- loads (all fast HWDGE; chain inputs first) ----------------
            def load(dram, shape, tag, dtype=f32r, chunks=1):
                r = w_pool.tile(shape, dtype, tag=tag)
                src = dram.ap().rearrange("p (t b) -> p t b", t=shape[1])
                for ch in range(chunks):
                    lo = shape[1] * ch // chunks
                    hi = shape[1] * (ch + 1) // chunks
                    nc.sync.dma_start(r[:, lo:hi, :], src[:, lo:hi, :])
                return r

            # d0/t0 gate the squaring chain: keep their triggers alone on the
            # Sync queue, interleaved per k-tile chunk; everything else goes
            # through the Scalar HWDGE queue so it never delays the chain.
            d_cur = w_pool.tile([P, DT, DZ], f32r, tag="d0")
            t_cur = w_pool.tile([P, DT, DZ], f32r, tag="t0")
            d0_src = d0_d.ap().rearrange("p (t b) -> p t b", t=DT)
            t0_src = t0_d.ap().rearrange("p (t b) -> p t b", t=DT)
            for ch in range(0, DT, 2):
                nc.sync.dma_start(d_cur[:, ch:ch + 2, :], d0_src[:, ch:ch + 2, :])
                nc.sync.dma_start(t_cur[:, ch:ch + 2, :], t0_src[:, ch:ch + 2, :])

            def load(dram, shape, tag, dtype=f32r):
                r = w_pool.tile(shape, dtype, tag=tag)
                nc.scalar.dma_start(
                    r[:], dram.ap().rearrange("p (t b) -> p t b", t=shape[1]))
                return r

            xt_r = load(xtr_d, [P, DT, BC], "xtr")
            xt = load(xt_d, [P, DT, BC], "xt", dtype=f32)
            w1t = load(w1t_d, [P, DT, H], "w1t")
            w2t = load(w2t_d, [P, HT, DY], "w2t")

            b1t = const_pool.tile([P, HT], f32, tag="b1t")
            nc.scalar.dma_start(b1t[:], b1t_d.ap())
            b2t = const_pool.tile([DY, 1], f32, tag="b2t")
            nc.scalar.dma_start(b2t[:], b2t_d.ap())
            ident = const_pool.tile([P, P], f32, tag="ident")
            nc.scalar.dma_start(ident[:], ident_d.ap())

            # ---- binary exponentiation on the deviation chain --------------
            acc = xt_r          # zT accumulator, fp32r [P, DT, BC]
            acc_f32 = xt        # exact fp32 twin, used for the fused +acc add

            def apply_T(t_tile, acc_r, acc_exact):
                """acc <- acc + D @ acc   (W^(2^k) application)."""
                new_r = acc_pool.tile([P, DT, BC], f32r, tag="acc")

                def evict(mt, ps):
                    nc.vector.scalar_tensor_tensor(
                        new_r[:, mt, :], acc_exact[:, mt, :], 1.0, ps[:],
                        op0=mult, op1=add)

                _emit_mm_set(nc, psum_pool, t_tile, acc_r, evict)
                return new_r, new_r

            def square_level(d_tile, t_tile, with_d):
                """One chain level: T' = 2T + T@T (and D' = 2D + D@D when
                still needed). T and D sets interleave per output tile mt so
                that evictions for k-tile kt land early — the next level's
                MM(mt, kt) only needs the kt-th evictions, so levels overlap
                with no PE bubble."""
                t_new = chain_pool.tile([P, DT, DZ], f32r, tag="T")
                if with_d:
                    d_new = chain_pool.tile([P, DT, DZ], f32r, tag="D")
                else:
                    d_new = None
                for mt in range(DT):
                    ps_t = psum_pool.tile([P, BC], f32, tag="ps")
                    for kt in range(DT):
                        nc.tensor.matmul(
                            ps_t[:], d_tile[:, kt, mt * P:(mt + 1) * P],
                            t_tile[:, kt, :], start=(kt == 0), stop=(kt == DT - 1))
                    nc.vector.scalar_tensor_tensor(
                        t_new[:, mt, :], t_tile[:, mt, :], 2.0, ps_t[:],
                        op0=mult, op1=add)
                    if with_d:
                        ps_d = psum_pool.tile([P, BC], f32, tag="ps")
                        for kt in range(DT):
                            nc.tensor.matmul(
                                ps_d[:], t_tile[:, kt, mt * P:(mt + 1) * P],
                                d_tile[:, kt, :], start=(kt == 0), stop=(kt == DT - 1))
                        nc.vector.scalar_tensor_tensor(
                            d_new[:, mt, :], d_tile[:, mt, :], 2.0, ps_d[:],
                            op0=mult, op1=add)
                return t_new, d_new

            if n > 0:
                maxbit = n.bit_length() - 1
                if (n >> 0) & 1:
                    acc, acc_f32 = apply_T(t_cur, acc, acc_f32)
                for k in range(1, maxbit + 1):
                    t_cur, d_cur = square_level(d_cur, t_cur, with_d=(k < maxbit))
                    if (n >> k) & 1:
                        acc, acc_f32 = apply_T(t_cur, acc, acc_f32)

            zt = acc  # fp32r [P, DT, BC]

            # ---- MLP: hT = relu(W1 @ z + b1); yT = W2 @ h + b2 -------------
            # Layer-2 accumulation MMs interleave with layer-1 so the tail
            # after the last h-tile is just one MM + bias + transpose.
            ht = act_pool.tile([P, HT, BC], f32r, tag="ht")
            ps_y = psum_y_pool.tile([DY, BC], f32, tag="psy")
            for mt in range(HT):
                ps = psum_pool.tile([P, BC], f32, tag="ps")
                for kt in range(DT):
                    nc.tensor.matmul(
                        ps[:], w1t[:, kt, mt * P:(mt + 1) * P], zt[:, kt, :],
                        start=(kt == 0), stop=(kt == DT - 1))
                nc.scalar.activation(
                    ht[:, mt, :], ps[:], mybir.ActivationFunctionType.Relu,
                    bias=b1t[:, mt:mt + 1])
                nc.tensor.matmul(ps_y[:], w2t[:, mt, :], ht[:, mt, :],
                                 start=(mt == 0), stop=(mt == HT - 1))
            ytb = out_pool.tile([DY, BC], f32, tag="ytb")
            nc.scalar.activation(ytb[:], ps_y[:],
                                 mybir.ActivationFunctionType.Identity,
                                 bias=b2t[:])

            # ---- transpose yT -> y and store -------------------------------
            y_sb = out_pool.tile([P, BC // P, DY], f32, tag="ysb")
            for bt in range(BC // P):
                ps_t = psum_y_pool.tile([P, DY], f32, tag="psy")
                nc.tensor.transpose(
                    ps_t[:], ytb[:, bt * P:(bt + 1) * P], ident[:DY, :DY])
                nc.vector.tensor_copy(y_sb[:, bt, :], ps_t[:])
            nc.sync.dma_start(
                y_d.ap().rearrange("(bt p) j -> p bt j", p=P), y_sb[:])

    nc.compile()
    return nc


def _tiles_pk(m: np.ndarray) -> np.ndarray:
    """[nt*128, C] -> [128, nt*C] partition-tiled layout (row r = kt*128+p)."""
    nt = m.shape[0] // P
    return np.ascontiguousarray(m.reshape(nt, P, -1).swapaxes(0, 1)).reshape(P, -1)


def kernel(x, A, W1, b1, W2, b2, n_steps) -> np.ndarray:
    x = np.asarray(x, dtype=np.float32)
    A = np.asarray(A, dtype=np.float32)
    W1 = np.asarray(W1, dtype=np.float32)
    b1 = np.asarray(b1, dtype=np.float32)
    W2 = np.asarray(W2, dtype=np.float32)
    b2 = np.asarray(b2, dtype=np.float32)
    n = int(np.asarray(n_steps))

    if n not in _BUILD_CACHE:
        _BUILD_CACHE[n] = _build(n)
    nc = _BUILD_CACHE[n]

    dt = np.float32(1.0 / n) if n > 0 else np.float32(0.0)
    d0 = _tiles_pk((dt * A).astype(np.float32))
    t0 = _tiles_pk((dt * A.T).astype(np.float32))
    w1t = _tiles_pk(np.ascontiguousarray(W1.T))           # [512, 2048]
    w2t = _tiles_pk(np.ascontiguousarray(W2.T))           # [2048, 10]
    b1t = np.ascontiguousarray(b1.reshape(HT, P).T)       # [128, 16]
    b2t = np.ascontiguousarray(b2.reshape(DY, 1))
    ident = np.eye(P, dtype=np.float32)

    in_maps = []
    for c in range(NCORES):
        xs = x[c * BC:(c + 1) * BC, :]                    # [512, 512]
        xt = _tiles_pk(np.ascontiguousarray(xs.T))        # [128, 4*512]
        in_maps.append({
            "xt": xt, "xtr": xt, "d0": d0, "t0": t0, "w1t": w1t, "b1t": b1t,
            "w2t": w2t, "b2t": b2t, "ident": ident,
        })

    trace = bool(os.environ.get("BASS_KERNEL_TRACE"))
    kwargs = {}
    if trace:
        kwargs = {"trace": True, "trace_cores": [0]}
    res = run_bass_kernel_spmd(nc, in_maps, list(range(NCORES)), **kwargs)
    if trace and res.exec_time_ns is not None:
        print(f"HW exec time: {res.exec_time_ns} ns")

    y = np.concatenate([res.results[c]["y"] for c in range(NCORES)], axis=0)
    return y.astype(np.float32)


# revision 14
# speedup vs baseline: 1.1801x; 1.1801x over previous
"""Trainium2 Bass kernel for MatrixOdeGradientDescentModel.

Reference computation (B=4096, DZ=512, H=2048, DY=10, n_steps=64):
    z = x; repeat n_steps: z += dt * z @ A.T          (dt = 1/n_steps)
    y = relu(z @ W1.T + b1) @ W2.T + b2

Algebraic rewrite: the Euler loop is linear, so
    z_final = x @ (M^T)^n  with  M = I + dt*A.
We compute W := M^T = I + dt*A^T by repeated squaring on the *deviation*
D_k := W^(2^k) - I (avoids precision loss from the identity's magnitude):
    D_{k+1} = 2*D_k + D_k @ D_k
maintaining the pair (D_k, T_k=D_k^T) so no on-device transposes are needed:
    D@D = matmul(lhsT=T, rhs=D),   (D@D)^T = matmul(lhsT=D, rhs=T)
then zT = xT + D_chain applied to xT per set bit of n (binary exponentiation).

Sharding: data-parallel over batch. Each of the 8 cores gets 512 rows of x;
A/W1/W2 replicated; no cross-core communication.

Matmuls run in float32r (TF32-like, 4x faster than fp32 on the PE) with fp32
PSUM accumulation; the error-compensated deviation chain keeps the end-to-end
relative error at the ~1e-4 level.
"""

import os

import numpy as np

import concourse.bacc as bacc
import concourse.mybir as mybir
import concourse.tile as tile
from concourse.bass_utils import run_bass_kernel_spmd

P = 128
B, DZ, H, DY = 4096, 512, 2048, 10
NCORES = 8
BC = B // NCORES          # 512 rows per core
DT = DZ // P              # 4 k-tiles over DZ
HT = H // P               # 16 m-tiles over H

f32 = mybir.dt.float32
f32r = mybir.dt.float32r

_BUILD_CACHE = {}


def _emit_mm_set(nc, psum_pool, lhsT_tile, rhs_tile, evict, n_mt=DT):
    """One [512,512]-ish matmul set: for each output row-block mt, accumulate
    over DT k-tiles into PSUM and call evict(mt, psum_ap)."""
    for mt in range(n_mt):
        ps = psum_pool.tile([P, BC], f32, tag="ps")
        for kt in range(DT):
            nc.tensor.matmul(
                ps[:],
                lhsT_tile[:, kt, mt * P:(mt + 1) * P],
                rhs_tile[:, kt, :],
                start=(kt == 0),
                stop=(kt == DT - 1),
            )
        evict(mt, ps)


def _build(n_steps: int):
    """Build + compile the Bass module for a given n_steps."""
    n = int(n_steps)
    assert n >= 0
    nc = bacc.Bacc("TRN2", target_bir_lowering=False, debug=False,
                   enable_asserts=False, num_devices=NCORES)

    # f32r-declared DRAM inputs carry raw fp32 bytes; the PE rounds internally
    # (verified bit-identical to an explicit cast) so plain HWDGE DMA works.
    xt_d = nc.dram_tensor("xt", [P, DT * BC], f32, kind="ExternalInput")
    xtr_d = nc.dram_tensor("xtr", [P, DT * BC], f32r, kind="ExternalInput")
    d0_d = nc.dram_tensor("d0", [P, DT * DZ], f32r, kind="ExternalInput")
    t0_d = nc.dram_tensor("t0", [P, DT * DZ], f32r, kind="ExternalInput")
    w1t_d = nc.dram_tensor("w1t", [P, DT * H], f32r, kind="ExternalInput")
    b1t_d = nc.dram_tensor("b1t", [P, HT], f32, kind="ExternalInput")
    w2t_d = nc.dram_tensor("w2t", [P, HT * DY], f32r, kind="ExternalInput")
    b2t_d = nc.dram_tensor("b2t", [DY, 1], f32, kind="ExternalInput")
    ident_d = nc.dram_tensor("ident", [P, P], f32, kind="ExternalInput")
    y_d = nc.dram_tensor("y", [BC, DY], f32, kind="ExternalOutput")

    mult = mybir.AluOpType.mult
    add = mybir.AluOpType.add

    with tile.TileContext(nc) as tc:
        napply = bin(n).count("1") if n > 0 else 0
        with (
            tc.tile_pool(name="const", bufs=1) as const_pool,
            tc.tile_pool(name="weights", bufs=1) as w_pool,
            tc.tile_pool(name="chain", bufs=2) as chain_pool,
            tc.tile_pool(name="accp", bufs=min(napply, 2) or 1) as acc_pool,
            tc.tile_pool(name="acts", bufs=1) as act_pool,
            tc.tile_pool(name="out", bufs=2) as out_pool,
            tc.tile_pool(name="psum", bufs=7, space="PSUM") as psum_pool,
            tc.tile_pool(name="psum_y", bufs=1, space="PSUM") as psum_y_pool,
        ):
            # ---- loads (all fast HWDGE; chain inputs first) ----------------
            def load(dram, shape, tag, dtype=f32r, chunks=1):
                r = w_pool.tile(shape, dtype, tag=tag)
                src = dram.ap().rearrange("p (t b) -> p t b", t=shape[1])
                for ch in range(chunks):
                    lo = shape[1] * ch // chunks
                    hi = shape[1] * (ch + 1) // chunks
                    nc.sync.dma_start(r[:, lo:hi, :], src[:, lo:hi, :])
                return r

            # All loads go through one trigger queue (Sync) in priority order:
            # the DMA rings are FIFO, so d0/t0 — which gate the squaring
            # chain — must be enqueued before the bulk weight loads.
            d_cur = w_pool.tile([P, DT, DZ], f32r, tag="d0")
            t_cur = w_pool.tile([P, DT, DZ], f32r, tag="t0")
            d0_src = d0_d.ap().rearrange("p (t b) -> p t b", t=DT)
            t0_src = t0_d.ap().rearrange("p (t b) -> p t b", t=DT)
            for ch in range(0, DT, 2):
                nc.sync.dma_start(d_cur[:, ch:ch + 2, :], d0_src[:, ch:ch + 2, :])
                nc.sync.dma_start(t_cur[:, ch:ch + 2, :], t0_src[:, ch:ch + 2, :])

            def load(dram, shape, tag, dtype=f32r):
                r = w_pool.tile(shape, dtype, tag=tag)
                nc.sync.dma_start(
                    r[:], dram.ap().rearrange("p (t b) -> p t b", t=shape[1]))
                return r

            xt_r = load(xtr_d, [P, DT, BC], "xtr")
            xt = load(xt_d, [P, DT, BC], "xt", dtype=f32)
            w1t = load(w1t_d, [P, DT, H], "w1t")
            w2t = load(w2t_d, [P, HT, DY], "w2t")

            b1t = const_pool.tile([P, HT], f32, tag="b1t")
            nc.sync.dma_start(b1t[:], b1t_d.ap())
            b2t = const_pool.tile([DY, 1], f32, tag="b2t")
            nc.sync.dma_start(b2t[:], b2t_d.ap())
            ident = const_pool.tile([P, P], f32, tag="ident")
            nc.sync.dma_start(ident[:], ident_d.ap())

            # ---- binary exponentiation on the deviation chain --------------
            acc = xt_r          # zT accumulator, fp32r [P, DT, BC]
            acc_f32 = xt        # exact fp32 twin, used for the fused +acc add

            def apply_T(t_tile, acc_r, acc_exact):
                """acc <- acc + D @ acc   (W^(2^k) application)."""
                new_r = acc_pool.tile([P, DT, BC], f32r, tag="acc")

                def evict(mt, ps):
                    nc.vector.scalar_tensor_tensor(
                        new_r[:, mt, :], acc_exact[:, mt, :], 1.0, ps[:],
                        op0=mult, op1=add)

                _emit_mm_set(nc, psum_pool, t_tile, acc_r, evict)
                return new_r, new_r

            def square_level(d_tile, t_tile, with_d):
                """One chain level: T' = 2T + T@T (and D' = 2D + D@D when
                still needed). T and D sets interleave per output tile mt so
                that evictions for k-tile kt land early — the next level's
                MM(mt, kt) only needs the kt-th evictions, so levels overlap
                with no PE bubble."""
                t_new = chain_pool.tile([P, DT, DZ], f32r, tag="T")
                if with_d:
                    d_new = chain_pool.tile([P, DT, DZ], f32r, tag="D")
                else:
                    d_new = None
                for mt in range(DT):
                    ps_t = psum_pool.tile([P, BC], f32, tag="ps")
                    for kt in range(DT):
                        nc.tensor.matmul(
                            ps_t[:], d_tile[:, kt, mt * P:(mt + 1) * P],
                            t_tile[:, kt, :], start=(kt == 0), stop=(kt == DT - 1))
                    nc.vector.scalar_tensor_tensor(
                        t_new[:, mt, :], t_tile[:, mt, :], 2.0, ps_t[:],
                        op0=mult, op1=add)
                    if with_d:
                        ps_d = psum_pool.tile([P, BC], f32, tag="ps")
                        for kt in range(DT):
                            nc.tensor.matmul(
                                ps_d[:], t_tile[:, kt, mt * P:(mt + 1) * P],
                                d_tile[:, kt, :], start=(kt == 0), stop=(kt == DT - 1))
                        nc.vector.scalar_tensor_tensor(
                            d_new[:, mt, :], d_tile[:, mt, :], 2.0, ps_d[:],
                            op0=mult, op1=add)
                return t_new, d_new

            if n > 0:
                maxbit = n.bit_length() - 1
                if (n >> 0) & 1:
                    acc, acc_f32 = apply_T(t_cur, acc, acc_f32)
                for k in range(1, maxbit + 1):
                    t_cur, d_cur = square_level(d_cur, t_cur, with_d=(k < maxbit))
                    if (n >> k) & 1:
                        acc, acc_f32 = apply_T(t_cur, acc, acc_f32)

            zt = acc  # fp32r [P, DT, BC]

            # ---- MLP: hT = relu(W1 @ z + b1); yT = W2 @ h + b2 -------------
            # Layer-2 accumulation MMs interleave with layer-1 so the tail
            # after the last h-tile is just one MM + bias + transpose.
            ht = act_pool.tile([P, HT, BC], f32r, tag="ht")
            ps_y = psum_y_pool.tile([DY, BC], f32, tag="psy")
            for mt in range(HT):
                ps = psum_pool.tile([P, BC], f32, tag="ps")
                for kt in range(DT):
                    nc.tensor.matmul(
                        ps[:], w1t[:, kt, mt * P:(mt + 1) * P], zt[:, kt, :],
                        start=(kt == 0), stop=(kt == DT - 1))
                nc.scalar.activation(
                    ht[:, mt, :], ps[:], mybir.ActivationFunctionType.Relu,
                    bias=b1t[:, mt:mt + 1])
                nc.tensor.matmul(ps_y[:], w2t[:, mt, :], ht[:, mt, :],
                                 start=(mt == 0), stop=(mt == HT - 1))
            ytb = out_pool.tile([DY, BC], f32, tag="ytb")
            nc.scalar.activation(ytb[:], ps_y[:],
                                 mybir.ActivationFunctionType.Identity,
                                 bias=b2t[:])

            # ---- transpose yT -> y and store -------------------------------
            y_sb = out_pool.tile([P, BC // P, DY], f32, tag="ysb")
            for bt in range(BC // P):
                ps_t = psum_y_pool.tile([P, DY], f32, tag="psy")
                nc.tensor.transpose(
                    ps_t[:], ytb[:, bt * P:(bt + 1) * P], ident[:DY, :DY])
                nc.vector.tensor_copy(y_sb[:, bt, :], ps_t[:])
            nc.sync.dma_start(
                y_d.ap().rearrange("(bt p) j -> p bt j", p=P), y_sb[:])

    nc.compile()
    return nc


def _tiles_pk(m: np.ndarray) -> np.ndarray:
    """[nt*128, C] -> [128, nt*C] partition-tiled layout (row r = kt*128+p)."""
    nt = m.shape[0] // P
    return np.ascontiguousarray(m.reshape(nt, P, -1).swapaxes(0, 1)).reshape(P, -1)


def kernel(x, A, W1, b1, W2, b2, n_steps) -> np.ndarray:
    x = np.asarray(x, dtype=np.float32)
    A = np.asarray(A, dtype=np.float32)
    W1 = np.asarray(W1, dtype=np.float32)
    b1 = np.asarray(b1, dtype=np.float32)
    W2 = np.asarray(W2, dtype=np.float32)
    b2 = np.asarray(b2, dtype=np.float32)
    n = int(np.asarray(n_steps))

    if n not in _BUILD_CACHE:
        _BUILD_CACHE[n] = _build(n)
    nc = _BUILD_CACHE[n]

    dt = np.float32(1.0 / n) if n > 0 else np.float32(0.0)
    d0 = _tiles_pk((dt * A).astype(np.float32))
    t0 = _tiles_pk((dt * A.T).astype(np.float32))
    w1t = _tiles_pk(np.ascontiguousarray(W1.T))           # [512, 2048]
    w2t = _tiles_pk(np.ascontiguousarray(W2.T))           # [2048, 10]
    b1t = np.ascontiguousarray(b1.reshape(HT, P).T)       # [128, 16]
    b2t = np.ascontiguousarray(b2.reshape(DY, 1))
    ident = np.eye(P, dtype=np.float32)

    in_maps = []
    for c in range(NCORES):
        xs = x[c * BC:(c + 1) * BC, :]                    # [512, 512]
        xt = _tiles_pk(np.ascontiguousarray(xs.T))        # [128, 4*512]
        in_maps.append({
            "xt": xt, "xtr": xt, "d0": d0, "t0": t0, "w1t": w1t, "b1t": b1t,
            "w2t": w2t, "b2t": b2t, "ident": ident,
        })

    trace = bool(os.environ.get("BASS_KERNEL_TRACE"))
    kwargs = {}
    if trace:
        kwargs = {"trace": True, "trace_cores": [0]}
    res = run_bass_kernel_spmd(nc, in_maps, list(range(NCORES)), **kwargs)
    if trace and res.exec_time_ns is not None:
        print(f"HW exec time: {res.exec_time_ns} ns")

    y = np.concatenate([res.results[c]["y"] for c in range(NCORES)], axis=0)
    return y.astype(np.float32)


# revision 18
# speedup vs baseline: 1.2028x; 1.0193x over previous
"""Trainium2 Bass kernel for MatrixOdeGradientDescentModel.

Reference computation (B=4096, DZ=512, H=2048, DY=10, n_steps=64):
    z = x; repeat n_steps: z += dt * z @ A.T          (dt = 1/n_steps)
    y = relu(z @ W1.T + b1) @ W2.T + b2

Algebraic rewrite: the Euler loop is linear, so
    z_final = x @ (M^T)^n  with  M = I + dt*A.
We compute W := M^T = I + dt*A^T by repeated squaring on the *deviation*
D_k := W^(2^k) - I (avoids precision loss from the identity's magnitude):
    D_{k+1} = 2*D_k + D_k @ D_k
maintaining the pair (D_k, T_k=D_k^T) so no on-device transposes are needed:
    D@D = matmul(lhsT=T, rhs=D),   (D@D)^T = matmul(lhsT=D, rhs=T)
then zT = xT + D_chain applied to xT per set bit of n (binary exponentiation).

Sharding: data-parallel over batch. Each of the 8 cores gets 512 rows of x;
A/W1/W2 replicated; no cross-core communication.

Matmuls run in float32r (TF32-like, 4x faster than fp32 on the PE) with fp32
PSUM accumulation; the error-compensated deviation chain keeps the end-to-end
relative error at the ~1e-4 level.
"""

import os

import numpy as np

import concourse.bacc as bacc
import concourse.mybir as mybir
import concourse.tile as tile
from concourse.bass_utils import run_bass_kernel_spmd

P = 128
B, DZ, H, DY = 4096, 512, 2048, 10
NCORES = 8
BC = B // NCORES          # 512 rows per core
DT = DZ // P              # 4 k-tiles over DZ
HT = H // P               # 16 m-tiles over H

f32 = mybir.dt.float32
f32r = mybir.dt.float32r

_BUILD_CACHE = {}


def _emit_mm_set(nc, psum_pool, lhsT_tile, rhs_tile, evict, n_mt=DT):
    """One [512,512]-ish matmul set: for each output row-block mt, accumulate
    over DT k-tiles into PSUM and call evict(mt, psum_ap)."""
    for mt in range(n_mt):
        ps = psum_pool.tile([P, BC], f32, tag="ps")
        for kt in range(DT):
            nc.tensor.matmul(
                ps[:],
                lhsT_tile[:, kt, mt * P:(mt + 1) * P],
                rhs_tile[:, kt, :],
                start=(kt == 0),
                stop=(kt == DT - 1),
            )
        evict(mt, ps)


def _build(n_steps: int):
    """Build + compile the Bass module for a given n_steps."""
    n = int(n_steps)
    assert n >= 0
    nc = bacc.Bacc("TRN2", target_bir_lowering=False, debug=False,
                   enable_asserts=False, num_devices=NCORES)

    # f32r-declared DRAM inputs carry raw fp32 bytes; the PE rounds internally
    # (verified bit-identical to an explicit cast) so plain HWDGE DMA works.
    xt_d = nc.dram_tensor("xt", [P, DT * BC], f32, kind="ExternalInput")
    xtr_d = nc.dram_tensor("xtr", [P, DT * BC], f32r, kind="ExternalInput")
    d0_d = nc.dram_tensor("d0", [P, DT * DZ], f32r, kind="ExternalInput")
    t0_d = nc.dram_tensor("t0", [P, DT * DZ], f32r, kind="ExternalInput")
    w1t_d = nc.dram_tensor("w1t", [P, DT * H], f32r, kind="ExternalInput")
    b1t_d = nc.dram_tensor("b1t", [P, HT], f32, kind="ExternalInput")
    w2t_d = nc.dram_tensor("w2t", [P, HT * DY], f32r, kind="ExternalInput")
    b2t_d = nc.dram_tensor("b2t", [DY, 1], f32, kind="ExternalInput")
    ident_d = nc.dram_tensor("ident", [P, P], f32, kind="ExternalInput")
    y_d = nc.dram_tensor("y", [BC, DY], f32, kind="ExternalOutput")

    mult = mybir.AluOpType.mult
    add = mybir.AluOpType.add

    with tile.TileContext(nc) as tc:
        with (
            tc.tile_pool(name="const", bufs=1) as const_pool,
            tc.tile_pool(name="weights", bufs=1) as w_pool,
            tc.tile_pool(name="chain", bufs=2) as chain_pool,
            tc.tile_pool(name="accp", bufs=2) as acc_pool,
            tc.tile_pool(name="acts", bufs=1) as act_pool,
            tc.tile_pool(name="out", bufs=2) as out_pool,
            tc.tile_pool(name="psum", bufs=7, space="PSUM") as psum_pool,
            tc.tile_pool(name="psum_y", bufs=1, space="PSUM") as psum_y_pool,
        ):
            # ---- loads (all fast HWDGE; chain inputs first) ----------------
            def load(dram, shape, tag, dtype=f32r, chunks=1):
                r = w_pool.tile(shape, dtype, tag=tag)
                src = dram.ap().rearrange("p (t b) -> p t b", t=shape[1])
                for ch in range(chunks):
                    lo = shape[1] * ch // chunks
                    hi = shape[1] * (ch + 1) // chunks
                    nc.sync.dma_start(r[:, lo:hi, :], src[:, lo:hi, :])
                return r

            # All loads go through one trigger queue (Sync) in priority order:
            # the DMA rings are FIFO, so d0/t0 — which gate the squaring
            # chain — must be enqueued before the bulk weight loads.
            d_cur = w_pool.tile([P, DT, DZ], f32r, tag="d0")
            t_cur = w_pool.tile([P, DT, DZ], f32r, tag="t0")
            d0_src = d0_d.ap().rearrange("p (t b) -> p t b", t=DT)
            t0_src = t0_d.ap().rearrange("p (t b) -> p t b", t=DT)
            for lo, hi in ((0, 1), (1, 2), (2, DT)):
                nc.sync.dma_start(d_cur[:, lo:hi, :], d0_src[:, lo:hi, :])
                nc.sync.dma_start(t_cur[:, lo:hi, :], t0_src[:, lo:hi, :])

            def load(dram, shape, tag, dtype=f32r):
                r = w_pool.tile(shape, dtype, tag=tag)
                nc.sync.dma_start(
                    r[:], dram.ap().rearrange("p (t b) -> p t b", t=shape[1]))
                return r

            xt_r = load(xtr_d, [P, DT, BC], "xtr")
            xt = load(xt_d, [P, DT, BC], "xt", dtype=f32)
            w1t = load(w1t_d, [P, DT, H], "w1t")
            w2t = load(w2t_d, [P, HT, DY], "w2t")

            b1t = const_pool.tile([P, HT], f32, tag="b1t")
            nc.sync.dma_start(b1t[:], b1t_d.ap())
            b2t = const_pool.tile([DY, 1], f32, tag="b2t")
            nc.sync.dma_start(b2t[:], b2t_d.ap())
            ident = const_pool.tile([P, P], f32, tag="ident")
            nc.sync.dma_start(ident[:], ident_d.ap())

            # ---- binary exponentiation on the deviation chain --------------
            acc = xt_r          # zT accumulator, fp32r [P, DT, BC]
            acc_f32 = xt        # exact fp32 twin, used for the fused +acc add

            def apply_T(t_tile, acc_r, acc_exact):
                """acc <- acc + D @ acc   (W^(2^k) application)."""
                new_r = acc_pool.tile([P, DT, BC], f32r, tag="acc")

                def evict(mt, ps):
                    nc.vector.scalar_tensor_tensor(
                        new_r[:, mt, :], acc_exact[:, mt, :], 1.0, ps[:],
                        op0=mult, op1=add)

                _emit_mm_set(nc, psum_pool, t_tile, acc_r, evict)
                return new_r, new_r

            def square_level(d_tile, t_tile, with_d):
                """One chain level: T' = 2T + T@T (and D' = 2D + D@D when
                still needed). T and D sets interleave per output tile mt so
                that evictions for k-tile kt land early — the next level's
                MM(mt, kt) only needs the kt-th evictions, so levels overlap
                with no PE bubble."""
                t_new = chain_pool.tile([P, DT, DZ], f32r, tag="T")
                if with_d:
                    d_new = chain_pool.tile([P, DT, DZ], f32r, tag="D")
                else:
                    d_new = None
                for mt in range(DT):
                    ps_t = psum_pool.tile([P, BC], f32, tag="ps")
                    for kt in range(DT):
                        nc.tensor.matmul(
                            ps_t[:], d_tile[:, kt, mt * P:(mt + 1) * P],
                            t_tile[:, kt, :], start=(kt == 0), stop=(kt == DT - 1))
                    nc.vector.scalar_tensor_tensor(
                        t_new[:, mt, :], t_tile[:, mt, :], 2.0, ps_t[:],
                        op0=mult, op1=add)
                    if with_d:
                        ps_d = psum_pool.tile([P, BC], f32, tag="ps")
                        for kt in range(DT):
                            nc.tensor.matmul(
                                ps_d[:], t_tile[:, kt, mt * P:(mt + 1) * P],
                                d_tile[:, kt, :], start=(kt == 0), stop=(kt == DT - 1))
                        nc.vector.scalar_tensor_tensor(
                            d_new[:, mt, :], d_tile[:, mt, :], 2.0, ps_d[:],
                            op0=mult, op1=add)
                return t_new, d_new

            if n == 1:
                acc, acc_f32 = apply_T(t_cur, acc, acc_f32)
            elif n > 1:
                # Binary exponentiation; the top bit is applied as a *fused
                # double application* of T_{mb-1}:
                #   u = T^t @ acc ; z = acc + 2u + T^t @ u
                # which skips the last chain level entirely (T_mb and D_{mb-1}
                # sets are never built): 32 fewer matmuls at ~no accuracy cost.
                mb = n.bit_length() - 1
                for k in range(0, mb):
                    if (n >> k) & 1:
                        acc, acc_f32 = apply_T(t_cur, acc, acc_f32)
                    if k < mb - 1:
                        t_cur, d_cur = square_level(d_cur, t_cur,
                                                    with_d=(k + 1 < mb - 1))
                u = acc_pool.tile([P, DT, BC], f32r, tag="uacc")
                for mt in range(DT):
                    ps = psum_pool.tile([P, BC], f32, tag="ps")
                    for kt in range(DT):
                        nc.tensor.matmul(
                            ps[:], t_cur[:, kt, mt * P:(mt + 1) * P],
                            acc[:, kt, :], start=(kt == 0), stop=(kt == DT - 1))
                    nc.scalar.activation(
                        u[:, mt, :], ps[:], mybir.ActivationFunctionType.Copy)
                znew = acc_pool.tile([P, DT, BC], f32r, tag="acc")
                for mt in range(DT):
                    ps = psum_pool.tile([P, BC], f32, tag="ps")
                    for kt in range(DT):
                        nc.tensor.matmul(
                            ps[:], t_cur[:, kt, mt * P:(mt + 1) * P],
                            u[:, kt, :], start=(kt == 0), stop=(kt == DT - 1))
                    nc.vector.scalar_tensor_tensor(
                        ps[:], u[:, mt, :], 2.0, ps[:], op0=mult, op1=add)
                    nc.vector.scalar_tensor_tensor(
                        znew[:, mt, :], acc_f32[:, mt, :], 1.0, ps[:],
                        op0=mult, op1=add)
                acc = znew

            zt = acc  # fp32r [P, DT, BC]

            # ---- MLP: hT = relu(W1 @ z + b1); yT = W2 @ h + b2 -------------
            # Layer-2 accumulation MMs interleave with layer-1 so the tail
            # after the last h-tile is just one MM + bias + transpose.
            ht = act_pool.tile([P, HT, BC], f32r, tag="ht")
            ps_y = psum_y_pool.tile([DY, BC], f32, tag="psy")
            for mt in range(HT):
                ps = psum_pool.tile([P, BC], f32, tag="ps")
                for kt in range(DT):
                    nc.tensor.matmul(
                        ps[:], w1t[:, kt, mt * P:(mt + 1) * P], zt[:, kt, :],
                        start=(kt == 0), stop=(kt == DT - 1))
                nc.scalar.activation(
                    ht[:, mt, :], ps[:], mybir.ActivationFunctionType.Relu,
                    bias=b1t[:, mt:mt + 1])
                nc.tensor.matmul(ps_y[:], w2t[:, mt, :], ht[:, mt, :],
                                 start=(mt == 0), stop=(mt == HT - 1))
            ytb = out_pool.tile([DY, BC], f32, tag="ytb")
            nc.scalar.activation(ytb[:], ps_y[:],
                                 mybir.ActivationFunctionType.Identity,
                                 bias=b2t[:])

            # ---- transpose yT -> y and store -------------------------------
            y_sb = out_pool.tile([P, BC // P, DY], f32, tag="ysb")
            for bt in range(BC // P):
                ps_t = psum_y_pool.tile([P, DY], f32, tag="psy")
                nc.tensor.transpose(
                    ps_t[:], ytb[:, bt * P:(bt + 1) * P], ident[:DY, :DY])
                nc.vector.tensor_copy(y_sb[:, bt, :], ps_t[:])
            nc.sync.dma_start(
                y_d.ap().rearrange("(bt p) j -> p bt j", p=P), y_sb[:])

    nc.compile()
    return nc


def _tiles_pk(m: np.ndarray) -> np.ndarray:
    """[nt*128, C] -> [128, nt*C] partition-tiled layout (row r = kt*128+p)."""
    nt = m.shape[0] // P
    return np.ascontiguousarray(m.reshape(nt, P, -1).swapaxes(0, 1)).reshape(P, -1)


def kernel(x, A, W1, b1, W2, b2, n_steps) -> np.ndarray:
    x = np.asarray(x, dtype=np.float32)
    A = np.asarray(A, dtype=np.float32)
    W1 = np.asarray(W1, dtype=np.float32)
    b1 = np.asarray(b1, dtype=np.float32)
    W2 = np.asarray(W2, dtype=np.float32)
    b2 = np.asarray(b2, dtype=np.float32)
    n = int(np.asarray(n_steps))

    if n not in _BUILD_CACHE:
        _BUILD_CACHE[n] = _build(n)
    nc = _BUILD_CACHE[n]

    dt = np.float32(1.0 / n) if n > 0 else np.float32(0.0)
    d0 = _tiles_pk((dt * A).astype(np.float32))
    t0 = _tiles_pk((dt * A.T).astype(np.float32))
    w1t = _tiles_pk(np.ascontiguousarray(W1.T))           # [512, 2048]
    w2t = _tiles_pk(np.ascontiguousarray(W2.T))           # [2048, 10]
    b1t = np.ascontiguousarray(b1.reshape(HT, P).T)       # [128, 16]
    b2t = np.ascontiguousarray(b2.reshape(DY, 1))
    ident = np.eye(P, dtype=np.float32)

    in_maps = []
    for c in range(NCORES):
        xs = x[c * BC:(c + 1) * BC, :]                    # [512, 512]
        xt = _tiles_pk(np.ascontiguousarray(xs.T))        # [128, 4*512]
        in_maps.append({
            "xt": xt, "xtr": xt, "d0": d0, "t0": t0, "w1t": w1t, "b1t": b1t,
            "w2t": w2t, "b2t": b2t, "ident": ident,
        })

    trace = bool(os.environ.get("BASS_KERNEL_TRACE"))
    kwargs = {}
    if trace:
        kwargs = {"trace": True, "trace_cores": [0]}
    res = run_bass_kernel_spmd(nc, in_maps, list(range(NCORES)), **kwargs)
    if trace and res.exec_time_ns is not None:
        print(f"HW exec time: {res.exec_time_ns} ns")

    y = np.concatenate([res.results[c]["y"] for c in range(NCORES)], axis=0)
    return y.astype(np.float32)
